# revision 17
# baseline (speedup 1.0000x reference)
"""Trainium2 Bass kernel for a quantized shared-expert MLP (SwiGLU, int8 dynamic quant).

Computation (per reference):
  x [2,4096,4096] f32 -> flatten [8192, 4096] -> bf16
  per-token int8 dynamic quant of x; int8 gemm vs w_gate/w_up (per-channel int8);
  swiglu with +-10 clip -> bf16; per-token requant; int8 gemm vs w_down; f32 out.

Strategy: data-parallel over the 8192 tokens across 8 NeuronCores (1024
tokens/core), weights replicated, no collectives.  All matmuls run in bf16,
which is exact here: quantized values are integers in [-127,127] (exact in
bf16) and partial sums stay far below 2^24, so the PE's fp32 accumulation
matches the reference's integer gemm.

Per-core schedule: the 1024 tokens are processed as 2 waves x 4 token-tiles
of 128.  All tile pools are resident simultaneously (~191KB/partition) so
there is no SBUF-reuse serialization between phases; the Tile scheduler then
overlaps the activation-quant (DVE/ACT) of wave w+1 and the requant of wave
w under the matmul stream, keeping the PE dense at the bf16 roofline.
Weight scales for the down proj are folded into the (bf16) weights host-side;
gate/up scales are applied in the epilogue from a broadcast [P,I] f32 tile.
Rounding uses the fp32 magic-number trick (x + 1.5*2^23) - 1.5*2^23 ==
round-to-nearest-even, matching jnp.round exactly; the subtract half runs on
the scalar engine (activation Copy with bias=-MAGIC).
"""

import numpy as np
import ml_dtypes

H = 4096
I = 2048
P = 128
T = 1024           # tokens per core (8192 / 8)
N_CORES = 8
QMAX = 127.0
LIMIT = 10.0
MAGIC = 12582912.0  # 1.5 * 2**23: fp32 add/sub rounds to nearest-even integer

_CACHE = {}


def _build(tokens=T):
    import concourse.bass as bass
    import concourse.bacc as bacc
    import concourse.mybir as mybir
    from concourse import tile
    from contextlib import ExitStack

    f32 = mybir.dt.float32
    bf16 = mybir.dt.bfloat16
    X = mybir.AxisListType.X
    MAX = mybir.AluOpType.max
    MIN = mybir.AluOpType.min
    MULT = mybir.AluOpType.mult
    ADD = mybir.AluOpType.add
    SUB = mybir.AluOpType.subtract
    Copy = mybir.ActivationFunctionType.Copy
    Silu = mybir.ActivationFunctionType.Silu

    NT = tokens // P        # 8 token tiles
    W = 2                   # waves
    MT = NT // W            # 4 token tiles per wave
    KH = H // P             # 32 k-tiles for gate/up
    KI = I // P             # 16 k-tiles for down
    NB = 256                # free-dim block for all gemms
    NBI = I // NB           # 8 gate/up n-blocks
    NBD = H // NB           # 16 down n-blocks
    HH = H // 2

    nc = bacc.Bacc("TRN2", target_bir_lowering=False, debug=False)

    x_d = nc.dram_tensor("x", [tokens, H], bf16, kind="ExternalInput")
    # weights are pre-tiled host-side to [n, P, K, NB] so each block load is
    # one contiguous 16KB descriptor per partition (the naive column-block
    # gather from [H, I] needs 4096 512B descriptors and stalls the DMA queue
    # ~17us per block)
    wgt_d = nc.dram_tensor("wgt", [NBI, P, KH, NB], bf16, kind="ExternalInput")
    wut_d = nc.dram_tensor("wut", [NBI, P, KH, NB], bf16, kind="ExternalInput")
    wdt_d = nc.dram_tensor("wdt", [NBD, P, KI, NB], bf16, kind="ExternalInput")
    swg_d = nc.dram_tensor("swg", [1, I], f32, kind="ExternalInput")
    swu_d = nc.dram_tensor("swu", [1, I], f32, kind="ExternalInput")
    out_d = nc.dram_tensor("out", [tokens, H], f32, kind="ExternalOutput")

    with ExitStack() as ctx:
        tc = ctx.enter_context(tile.TileContext(nc))

        const_p = ctx.enter_context(tc.tile_pool(name="const", bufs=1))
        sc_p = ctx.enter_context(tc.tile_pool(name="sc", bufs=1))
        qT_p = ctx.enter_context(tc.tile_pool(name="qT", bufs=1))
        xt_p = ctx.enter_context(tc.tile_pool(name="xt", bufs=2))
        t1_p = ctx.enter_context(tc.tile_pool(name="t1", bufs=2))
        wg_p = ctx.enter_context(tc.tile_pool(name="wg", bufs=2))
        wu_p = ctx.enter_context(tc.tile_pool(name="wu", bufs=2))
        ep_p = ctx.enter_context(tc.tile_pool(name="ep", bufs=3))
        inter_p = ctx.enter_context(tc.tile_pool(name="inter", bufs=1))
        qiT_p = ctx.enter_context(tc.tile_pool(name="qiT", bufs=1))
        wd_p = ctx.enter_context(tc.tile_pool(name="wd", bufs=2))
        out_p = ctx.enter_context(tc.tile_pool(name="outp", bufs=4))
        ps_gu = ctx.enter_context(
            tc.tile_pool(name="psgu", bufs=2, space=bass.MemorySpace.PSUM))
        ps_d = ctx.enter_context(
            tc.tile_pool(name="psd", bufs=4, space=bass.MemorySpace.PSUM))

        swg_b = const_p.tile([P, I], f32, tag="swg_b")
        swu_b = const_p.tile([P, I], f32, tag="swu_b")

        # per-token-tile scale columns (one column per global tile g)
        mx = sc_p.tile([P, NT], f32, tag="mx")
        sx = sc_p.tile([P, NT], f32, tag="sx")     # x quant scale (= max/127, clamped)
        ix = sc_p.tile([P, NT], f32, tag="ix")     # 1 / sx
        mxi = sc_p.tile([P, NT], f32, tag="mxi")
        si = sc_p.tile([P, NT], f32, tag="si")     # inter quant scale
        ii = sc_p.tile([P, NT], f32, tag="ii")     # 1 / si
        r0 = sc_p.tile([P, NT], f32, tag="r0")     # reciprocal seed / NR temps
        r1 = sc_p.tile([P, NT], f32, tag="r1")

        def nr_recip(out_t, in_t, gc):
            # out = 1/in with one Newton step: r1 = r0*(2 - in*r0); the HW
            # reciprocal seed is not accurate enough for exact round() parity.
            nc.vector.reciprocal(r0[:, gc], in_t[:, gc])
            nc.vector.tensor_tensor(r1[:, gc], in_t[:, gc], r0[:, gc], op=MULT)
            nc.vector.tensor_scalar(r1[:, gc], r1[:, gc], -1.0, 2.0, op0=MULT, op1=ADD)
            nc.vector.tensor_tensor(out_t[:, gc], r0[:, gc], r1[:, gc], op=MULT)

        qT = [qT_p.tile([P, KH, P], bf16, tag=f"qT{m}", name=f"qT{m}")
              for m in range(MT)]
        inter = [inter_p.tile([P, I], bf16, tag=f"inter{m}", name=f"inter{m}")
                 for m in range(MT)]
        qiT = [qiT_p.tile([P, KI, P], bf16, tag=f"qiT{m}", name=f"qiT{m}")
               for m in range(MT)]

        gu_queue = []
        wd_queue = []

        def emit_gu_load(n):
            wgt_t = wg_p.tile([P, KH, NB], bf16, tag="wg", name=f"wg{n}")
            wut_t = wu_p.tile([P, KH, NB], bf16, tag="wu", name=f"wu{n}")
            nc.sync.dma_start(wgt_t[:], wgt_d[n])
            nc.sync.dma_start(wut_t[:], wut_d[n])
            gu_queue.append((wgt_t, wut_t))

        def emit_wd_load(n):
            wd_t = wd_p.tile([P, KI, NB], bf16, tag="wd", name=f"wd{n}")
            nc.sync.dma_start(wd_t[:], wdt_d[n])
            wd_queue.append(wd_t)

        def a_load(g, xdma):
            # load x tile g + abs-max reduce
            gc = slice(g, g + 1)
            xt = xt_p.tile([P, H], bf16, tag="xt", name=f"xt{g}")
            xdma.dma_start(xt[:], x_d[g * P:(g + 1) * P, :])
            nc.vector.tensor_reduce(mx[:, gc], xt[:], axis=X, op=MAX,
                                    apply_absolute_value=True)
            return xt

        def a_round(g, xt):
            # scale + magic-round the tile in place (quantized ints, bf16)
            gc = slice(g, g + 1)
            nc.vector.tensor_scalar(sx[:, gc], mx[:, gc], 1.0 / QMAX, 1e-8,
                                    op0=MULT, op1=MAX)
            nr_recip(ix, sx, gc)
            for h in range(2):
                hs = slice(h * HH, (h + 1) * HH)
                t1 = t1_p.tile([P, HH], f32, tag="t1h", name=f"t1a{g}_{h}")
                nc.vector.tensor_scalar(t1[:], xt[:, hs], ix[:, gc], MAGIC,
                                        op0=MULT, op1=ADD)
                nc.vector.tensor_scalar(xt[:, hs], t1[:], MAGIC, None, op0=SUB)

        def a_transpose(g, xt):
            nc.scalar.dma_start(qT[g % MT][:], xt[:], transpose=True)

        def c_m(w, m):
            # requant one inter tile (in place), transpose to I-major qiT
            g = w * MT + m
            gc = slice(g, g + 1)
            nc.vector.tensor_reduce(mxi[:, gc], inter[m][:], axis=X, op=MAX,
                                    apply_absolute_value=True)
            nc.vector.tensor_scalar(si[:, gc], mxi[:, gc], 1.0 / QMAX, 1e-8,
                                    op0=MULT, op1=MAX)
            nr_recip(ii, si, gc)
            t1 = t1_p.tile([P, HH], f32, tag="t1h", name=f"t1c{g}")
            nc.vector.tensor_scalar(t1[:], inter[m][:], ii[:, gc], MAGIC,
                                    op0=MULT, op1=ADD)
            nc.scalar.activation(inter[m][:], t1[:], Copy, bias=-MAGIC)
            nc.scalar.dma_start(qiT[m][:], inter[m][:], transpose=True)

        def b_phase(w, preloaded=1):
            # gate/up gemms + swiglu + clip -> inter (bf16); expects the first
            # `preloaded` blocks already in gu_queue.  On the last n iteration,
            # each m tile's requant chain is emitted right after its epilogue
            # so it runs on DVE/ACT while the PE finishes the remaining tiles.
            nxt = preloaded
            for n in range(NBI):
                nb = slice(n * NB, (n + 1) * NB)
                wgt_t, wut_t = gu_queue.pop(0)
                if nxt == n + 1 and nxt < NBI:
                    emit_gu_load(nxt)
                    nxt += 1
                for m in range(MT):
                    g = w * MT + m
                    gc = slice(g, g + 1)
                    pg = ps_gu.tile([P, NB], f32, tag="pg", name=f"pg{w}_{n}_{m}")
                    pu = ps_gu.tile([P, NB], f32, tag="pu", name=f"pu{w}_{n}_{m}")
                    for k in range(KH):
                        nc.tensor.matmul(pg[:], qT[m][:, k, :], wgt_t[:, k, :],
                                         start=(k == 0), stop=(k == KH - 1))
                        nc.tensor.matmul(pu[:], qT[m][:, k, :], wut_t[:, k, :],
                                         start=(k == 0), stop=(k == KH - 1))
                    gs = ep_p.tile([P, NB], f32, tag="gs", name=f"gs{w}_{n}_{m}")
                    us = ep_p.tile([P, NB], f32, tag="us", name=f"us{w}_{n}_{m}")
                    nc.vector.scalar_tensor_tensor(gs[:], pg[:], sx[:, gc],
                                                   swg_b[:, nb], op0=MULT, op1=MULT)
                    nc.vector.scalar_tensor_tensor(us[:], pu[:], sx[:, gc],
                                                   swu_b[:, nb], op0=MULT, op1=MULT)
                    nc.scalar.activation(gs[:], gs[:], Silu)
                    nc.vector.tensor_tensor(us[:], gs[:], us[:], op=MULT)
                    nc.vector.tensor_scalar(inter[m][:, nb], us[:], LIMIT, -LIMIT,
                                            op0=MIN, op1=MAX)
                    if n == NBI - 1:
                        c_m(w, m)

        def d_phase(w, hook=None):
            # down gemm (weights pre-scaled by s_wdown) -> out; expects block 0
            # already in wd_queue.  `hook` emits one deferred work piece per n
            # iteration (wave-1 quant under D0) so in-order engine queues never
            # accumulate a long run of foreign work ahead of the epilogues.
            for n in range(NBD):
                nb = slice(n * NB, (n + 1) * NB)
                wd_t = wd_queue.pop(0)
                if n + 1 < NBD:
                    emit_wd_load(n + 1)
                for m in range(MT):
                    g = w * MT + m
                    gc = slice(g, g + 1)
                    po = ps_d.tile([P, NB], f32, tag="po", name=f"po{w}_{n}_{m}")
                    for k in range(KI):
                        nc.tensor.matmul(po[:], qiT[m][:, k, :], wd_t[:, k, :],
                                         start=(k == 0), stop=(k == KI - 1))
                    ot = out_p.tile([P, NB], f32, tag="ot", name=f"ot{w}_{n}_{m}")
                    nc.scalar.activation(ot[:], po[:], Copy, scale=si[:, gc])
                    nc.sync.dma_start(out_d[g * P:(g + 1) * P, nb], ot[:])
                if hook:
                    hook(n)

        # ---- emission schedule (program order sets scheduler priority) ----
        # Preload the ACT function tables (Silu) off the critical path.
        dum = sc_p.tile([P, 1], f32, tag="dum")
        nc.vector.memset(dum[:], 0.0)
        nc.scalar.activation(dum[:], dum[:], Silu)

        xts = {}
        # wave-0 quant: x tiles split between the sync and gpsimd DMA paths,
        # tile-0's full chain emitted first so every engine's in-order queue
        # matches the data-ready order.
        xts[0] = a_load(0, nc.sync)
        emit_gu_load(0)
        xts[1] = a_load(1, nc.gpsimd)
        a_round(0, xts[0])
        a_transpose(0, xts[0])
        xts[2] = a_load(2, nc.sync)
        nc.sync.dma_start(swg_b[:], swg_d[0:1, :].broadcast_to([P, I]))
        nc.sync.dma_start(swu_b[:], swu_d[0:1, :].broadcast_to([P, I]))
        emit_gu_load(1)
        a_round(1, xts[1])
        a_transpose(1, xts[1])
        xts[3] = a_load(3, nc.gpsimd)
        a_round(2, xts[2])
        a_transpose(2, xts[2])
        a_round(3, xts[3])
        a_transpose(3, xts[3])

        b_phase(0, preloaded=2)  # last n iter also emits wave-0 requant chains
        emit_wd_load(0)          # prefetch down weights for D0
        emit_gu_load(0)          # prefetch gate/up block 0 for B1

        # wave-1 quant split into pieces, one per D0 n-iteration
        pieces = []
        for g in (4, 5, 6, 7):
            def mk_load(g=g):
                xts[g] = a_load(g, nc.gpsimd)
            def mk_round(g=g):
                a_round(g, xts[g])
            def mk_tr(g=g):
                a_transpose(g, xts[g])
            pieces += [mk_load, mk_round, mk_tr]

        def d0_hook(n):
            if n < len(pieces):
                pieces[n]()

        d_phase(0, hook=d0_hook)
        b_phase(1, preloaded=1)
        emit_wd_load(0)
        d_phase(1)

    if not nc.is_finalized():
        nc.finalize()
    return nc


def _tile4(wT, K, NBn, NB=256):
    # [K*128, NBn*NB] -> [NBn, 128, K, NB] so each device block load reads one
    # contiguous 16KB run per partition
    return np.ascontiguousarray(
        wT.reshape(K, P, NBn, NB).transpose(2, 1, 0, 3))


def _prep_inputs(x, w_gate, s_wgate, w_up, s_wup, w_down, s_wdown):
    bf16 = ml_dtypes.bfloat16
    x_flat = np.ascontiguousarray(x.reshape(-1, H)).astype(bf16)
    wgt = _tile4(w_gate.astype(bf16).T, H // P, I // 256)  # int-valued: exact
    wut = _tile4(w_up.astype(bf16).T, H // P, I // 256)
    wdt = _tile4(
        (w_down.astype(np.float32) * s_wdown.astype(np.float32)[:, None]).T
        .astype(bf16), I // P, H // 256)                   # fold s_wdown in
    swg = np.ascontiguousarray(s_wgate.reshape(1, I).astype(np.float32))
    swu = np.ascontiguousarray(s_wup.reshape(1, I).astype(np.float32))
    return x_flat, wgt, wut, wdt, swg, swu


def kernel(x, w_gate, s_wgate, w_up, s_wup, w_down, s_wdown,
           inv_gate, inv_up, inv_inter):
    from concourse.bass_utils import run_bass_kernel_spmd

    x_flat, wgt, wut, wdt, swg, swu = _prep_inputs(
        x, w_gate, s_wgate, w_up, s_wup, w_down, s_wdown)

    if "nc" not in _CACHE:
        _CACHE["nc"] = _build()
    nc = _CACHE["nc"]

    in_maps = []
    for c in range(N_CORES):
        in_maps.append({
            "x": np.ascontiguousarray(x_flat[c * T:(c + 1) * T]),
            "wgt": wgt, "wut": wut, "wdt": wdt,
            "swg": swg, "swu": swu,
        })
    res = run_bass_kernel_spmd(nc, in_maps, list(range(N_CORES)))
    _CACHE["last_results"] = res
    _CACHE["last_in_maps"] = in_maps
    out = np.concatenate([res.results[c]["out"] for c in range(N_CORES)], axis=0)
    return out.reshape(x.shape).astype(np.float32)


# revision 21
# speedup vs baseline: 1.1601x; 1.1601x over previous
"""Trainium2 Bass kernel for a quantized shared-expert MLP (SwiGLU, int8 dynamic quant).

Computation (per reference):
  x [2,4096,4096] f32 -> flatten [8192, 4096] -> bf16
  per-token int8 dynamic quant of x; int8 gemm vs w_gate/w_up (per-channel int8);
  swiglu with +-10 clip -> bf16; per-token requant; int8 gemm vs w_down; f32 out.

Strategy: data-parallel over the 8192 tokens across 8 NeuronCores (1024
tokens/core), weights replicated, no collectives.  All matmuls run in bf16,
which is exact here: quantized values are integers in [-127,127] (exact in
bf16) and partial sums stay far below 2^24, so the PE's fp32 accumulation
matches the reference's integer gemm.

Per-core schedule: the 1024 tokens are processed as 2 waves x 4 token-tiles
of 128.  All tile pools are resident simultaneously (~191KB/partition) so
there is no SBUF-reuse serialization between phases; the Tile scheduler then
overlaps the activation-quant (DVE/ACT) of wave w+1 and the requant of wave
w under the matmul stream, keeping the PE dense at the bf16 roofline.
Weight scales for the down proj are folded into the (bf16) weights host-side;
gate/up scales are applied in the epilogue from a broadcast [P,I] f32 tile.
Rounding uses the fp32 magic-number trick (x + 1.5*2^23) - 1.5*2^23 ==
round-to-nearest-even, matching jnp.round exactly; the subtract half runs on
the scalar engine (activation Copy with bias=-MAGIC).
"""

import numpy as np
import ml_dtypes

H = 4096
I = 2048
P = 128
T = 1024           # tokens per core (8192 / 8)
N_CORES = 8
QMAX = 127.0
LIMIT = 10.0
MAGIC = 12582912.0  # 1.5 * 2**23: fp32 add/sub rounds to nearest-even integer

_CACHE = {}


def _build(tokens=T):
    import concourse.bass as bass
    import concourse.bacc as bacc
    import concourse.mybir as mybir
    from concourse import tile
    from contextlib import ExitStack

    f32 = mybir.dt.float32
    bf16 = mybir.dt.bfloat16
    X = mybir.AxisListType.X
    MAX = mybir.AluOpType.max
    MIN = mybir.AluOpType.min
    MULT = mybir.AluOpType.mult
    ADD = mybir.AluOpType.add
    SUB = mybir.AluOpType.subtract
    Copy = mybir.ActivationFunctionType.Copy
    Silu = mybir.ActivationFunctionType.Silu

    NT = tokens // P        # 8 token tiles
    W = 2                   # waves
    MT = NT // W            # 4 token tiles per wave
    KH = H // P             # 32 k-tiles for gate/up
    KI = I // P             # 16 k-tiles for down
    NB = 256                # free-dim block for all gemms
    NBI = I // NB           # 8 gate/up n-blocks
    NBD = H // NB           # 16 down n-blocks
    HH = H // 2

    nc = bacc.Bacc("TRN2", target_bir_lowering=False, debug=False)

    x_d = nc.dram_tensor("x", [tokens, H], bf16, kind="ExternalInput")
    # weights are pre-tiled host-side to [n, q, P, K/q, NB] so each block load
    # is 4KB-contiguous descriptors: the naive column-block gather from [H, I]
    # needs 4096 512B descriptors and stalls the DMA queue ~17us per block,
    # while full-16KB-per-partition runs monopolize single SBUF partitions
    # long enough to stall concurrent PE reads (~20% matmul slowdown).
    Q = 4
    wgt_d = nc.dram_tensor("wgt", [NBI, Q, P, KH // Q, NB], bf16,
                           kind="ExternalInput")
    wut_d = nc.dram_tensor("wut", [NBI, Q, P, KH // Q, NB], bf16,
                           kind="ExternalInput")
    wdt_d = nc.dram_tensor("wdt", [NBD, Q, P, KI // Q, NB], bf16,
                           kind="ExternalInput")
    swg_d = nc.dram_tensor("swg", [1, I], f32, kind="ExternalInput")
    swu_d = nc.dram_tensor("swu", [1, I], f32, kind="ExternalInput")
    out_d = nc.dram_tensor("out", [tokens, H], f32, kind="ExternalOutput")

    with ExitStack() as ctx:
        tc = ctx.enter_context(tile.TileContext(nc))

        const_p = ctx.enter_context(tc.tile_pool(name="const", bufs=1))
        sc_p = ctx.enter_context(tc.tile_pool(name="sc", bufs=1))
        qT_p = ctx.enter_context(tc.tile_pool(name="qT", bufs=1))
        xt_p = ctx.enter_context(tc.tile_pool(name="xt", bufs=2))
        t1_p = ctx.enter_context(tc.tile_pool(name="t1", bufs=2))
        wg_p = ctx.enter_context(tc.tile_pool(name="wg", bufs=2))
        wu_p = ctx.enter_context(tc.tile_pool(name="wu", bufs=2))
        ep_p = ctx.enter_context(tc.tile_pool(name="ep", bufs=3))
        inter_p = ctx.enter_context(tc.tile_pool(name="inter", bufs=1))
        qiT_p = ctx.enter_context(tc.tile_pool(name="qiT", bufs=1))
        wd_p = ctx.enter_context(tc.tile_pool(name="wd", bufs=2))
        out_p = ctx.enter_context(tc.tile_pool(name="outp", bufs=4))
        ps_gu = ctx.enter_context(
            tc.tile_pool(name="psgu", bufs=2, space=bass.MemorySpace.PSUM))
        ps_d = ctx.enter_context(
            tc.tile_pool(name="psd", bufs=4, space=bass.MemorySpace.PSUM))

        swg_b = const_p.tile([P, I], f32, tag="swg_b")
        swu_b = const_p.tile([P, I], f32, tag="swu_b")

        # per-token-tile scale columns (one column per global tile g)
        mx = sc_p.tile([P, NT], f32, tag="mx")
        sx = sc_p.tile([P, NT], f32, tag="sx")     # x quant scale (= max/127, clamped)
        ix = sc_p.tile([P, NT], f32, tag="ix")     # 1 / sx
        mxi = sc_p.tile([P, NT], f32, tag="mxi")
        si = sc_p.tile([P, NT], f32, tag="si")     # inter quant scale
        ii = sc_p.tile([P, NT], f32, tag="ii")     # 1 / si
        r0 = sc_p.tile([P, NT], f32, tag="r0")     # reciprocal seed / NR temps
        r1 = sc_p.tile([P, NT], f32, tag="r1")

        def nr_recip(out_t, in_t, gc):
            # out = 1/in with one Newton step: r1 = r0*(2 - in*r0); the HW
            # reciprocal seed is not accurate enough for exact round() parity.
            nc.vector.reciprocal(r0[:, gc], in_t[:, gc])
            nc.vector.tensor_tensor(r1[:, gc], in_t[:, gc], r0[:, gc], op=MULT)
            nc.vector.tensor_scalar(r1[:, gc], r1[:, gc], -1.0, 2.0, op0=MULT, op1=ADD)
            nc.vector.tensor_tensor(out_t[:, gc], r0[:, gc], r1[:, gc], op=MULT)

        qT = [qT_p.tile([P, KH, P], bf16, tag=f"qT{m}", name=f"qT{m}")
              for m in range(MT)]
        inter = [inter_p.tile([P, I], bf16, tag=f"inter{m}", name=f"inter{m}")
                 for m in range(MT)]
        qiT = [qiT_p.tile([P, KI, P], bf16, tag=f"qiT{m}", name=f"qiT{m}")
               for m in range(MT)]

        gu_queue = []
        wd_queue = []

        KHQ = KH // Q
        KIQ = KI // Q

        def emit_gu_load(n):
            wgt_t = wg_p.tile([P, KH, NB], bf16, tag="wg", name=f"wg{n}")
            wut_t = wu_p.tile([P, KH, NB], bf16, tag="wu", name=f"wu{n}")
            for q in range(Q):
                ks = slice(q * KHQ, (q + 1) * KHQ)
                nc.sync.dma_start(wgt_t[:, ks, :], wgt_d[n, q])
                nc.sync.dma_start(wut_t[:, ks, :], wut_d[n, q])
            gu_queue.append((wgt_t, wut_t))

        def emit_wd_load(n):
            wd_t = wd_p.tile([P, KI, NB], bf16, tag="wd", name=f"wd{n}")
            for q in range(Q):
                ks = slice(q * KIQ, (q + 1) * KIQ)
                nc.sync.dma_start(wd_t[:, ks, :], wdt_d[n, q])
            wd_queue.append(wd_t)

        def a_load(g, xdma):
            # load x tile g + abs-max reduce
            gc = slice(g, g + 1)
            xt = xt_p.tile([P, H], bf16, tag="xt", name=f"xt{g}")
            xdma.dma_start(xt[:], x_d[g * P:(g + 1) * P, :])
            nc.vector.tensor_reduce(mx[:, gc], xt[:], axis=X, op=MAX,
                                    apply_absolute_value=True)
            return xt

        def a_round(g, xt):
            # scale + magic-round the tile in place (quantized ints, bf16)
            gc = slice(g, g + 1)
            nc.vector.tensor_scalar(sx[:, gc], mx[:, gc], 1.0 / QMAX, 1e-8,
                                    op0=MULT, op1=MAX)
            nr_recip(ix, sx, gc)
            for h in range(2):
                hs = slice(h * HH, (h + 1) * HH)
                t1 = t1_p.tile([P, HH], f32, tag="t1h", name=f"t1a{g}_{h}")
                nc.vector.tensor_scalar(t1[:], xt[:, hs], ix[:, gc], MAGIC,
                                        op0=MULT, op1=ADD)
                nc.vector.tensor_scalar(xt[:, hs], t1[:], MAGIC, None, op0=SUB)

        def a_transpose(g, xt):
            nc.scalar.dma_start(qT[g % MT][:], xt[:], transpose=True)

        def c_m(w, m):
            # requant one inter tile (in place), transpose to I-major qiT
            g = w * MT + m
            gc = slice(g, g + 1)
            nc.vector.tensor_reduce(mxi[:, gc], inter[m][:], axis=X, op=MAX,
                                    apply_absolute_value=True)
            nc.vector.tensor_scalar(si[:, gc], mxi[:, gc], 1.0 / QMAX, 1e-8,
                                    op0=MULT, op1=MAX)
            nr_recip(ii, si, gc)
            t1 = t1_p.tile([P, HH], f32, tag="t1h", name=f"t1c{g}")
            nc.vector.tensor_scalar(t1[:], inter[m][:], ii[:, gc], MAGIC,
                                    op0=MULT, op1=ADD)
            nc.scalar.activation(inter[m][:], t1[:], Copy, bias=-MAGIC)
            nc.scalar.dma_start(qiT[m][:], inter[m][:], transpose=True)

        def b_phase(w, preloaded=1):
            # gate/up gemms + swiglu + clip -> inter (bf16); expects the first
            # `preloaded` blocks already in gu_queue.  On the last n iteration,
            # each m tile's requant chain is emitted right after its epilogue
            # so it runs on DVE/ACT while the PE finishes the remaining tiles.
            nxt = preloaded
            for n in range(NBI):
                nb = slice(n * NB, (n + 1) * NB)
                wgt_t, wut_t = gu_queue.pop(0)
                if nxt == n + 1 and nxt < NBI:
                    emit_gu_load(nxt)
                    nxt += 1
                for m in range(MT):
                    g = w * MT + m
                    gc = slice(g, g + 1)
                    pg = ps_gu.tile([P, NB], f32, tag="pg", name=f"pg{w}_{n}_{m}")
                    pu = ps_gu.tile([P, NB], f32, tag="pu", name=f"pu{w}_{n}_{m}")
                    for k in range(KH):
                        nc.tensor.matmul(pg[:], qT[m][:, k, :], wgt_t[:, k, :],
                                         start=(k == 0), stop=(k == KH - 1))
                        nc.tensor.matmul(pu[:], qT[m][:, k, :], wut_t[:, k, :],
                                         start=(k == 0), stop=(k == KH - 1))
                    gs = ep_p.tile([P, NB], f32, tag="gs", name=f"gs{w}_{n}_{m}")
                    us = ep_p.tile([P, NB], f32, tag="us", name=f"us{w}_{n}_{m}")
                    nc.vector.scalar_tensor_tensor(gs[:], pg[:], sx[:, gc],
                                                   swg_b[:, nb], op0=MULT, op1=MULT)
                    nc.vector.scalar_tensor_tensor(us[:], pu[:], sx[:, gc],
                                                   swu_b[:, nb], op0=MULT, op1=MULT)
                    nc.scalar.activation(gs[:], gs[:], Silu)
                    nc.vector.tensor_tensor(us[:], gs[:], us[:], op=MULT)
                    nc.vector.tensor_scalar(inter[m][:, nb], us[:], LIMIT, -LIMIT,
                                            op0=MIN, op1=MAX)
                    if n == NBI - 1:
                        c_m(w, m)

        def d_phase(w, hook=None):
            # down gemm (weights pre-scaled by s_wdown) -> out; expects block 0
            # already in wd_queue.  `hook` emits one deferred work piece per n
            # iteration (wave-1 quant under D0) so in-order engine queues never
            # accumulate a long run of foreign work ahead of the epilogues.
            for n in range(NBD):
                nb = slice(n * NB, (n + 1) * NB)
                wd_t = wd_queue.pop(0)
                if n + 1 < NBD:
                    emit_wd_load(n + 1)
                for m in range(MT):
                    g = w * MT + m
                    gc = slice(g, g + 1)
                    po = ps_d.tile([P, NB], f32, tag="po", name=f"po{w}_{n}_{m}")
                    for k in range(KI):
                        nc.tensor.matmul(po[:], qiT[m][:, k, :], wd_t[:, k, :],
                                         start=(k == 0), stop=(k == KI - 1))
                    ot = out_p.tile([P, NB], f32, tag="ot", name=f"ot{w}_{n}_{m}")
                    nc.scalar.activation(ot[:], po[:], Copy, scale=si[:, gc])
                    nc.sync.dma_start(out_d[g * P:(g + 1) * P, nb], ot[:])
                if hook:
                    hook(n)

        # ---- emission schedule (program order sets scheduler priority) ----
        # Preload the ACT function tables (Silu) off the critical path.
        dum = sc_p.tile([P, 1], f32, tag="dum")
        nc.vector.memset(dum[:], 0.0)
        nc.scalar.activation(dum[:], dum[:], Silu)

        xts = {}
        # wave-0 quant: x tiles split between the sync and gpsimd DMA paths,
        # tile-0's full chain emitted first so every engine's in-order queue
        # matches the data-ready order.
        xts[0] = a_load(0, nc.sync)
        emit_gu_load(0)
        xts[1] = a_load(1, nc.gpsimd)
        a_round(0, xts[0])
        a_transpose(0, xts[0])
        xts[2] = a_load(2, nc.sync)
        nc.sync.dma_start(swg_b[:], swg_d[0:1, :].broadcast_to([P, I]))
        nc.sync.dma_start(swu_b[:], swu_d[0:1, :].broadcast_to([P, I]))
        emit_gu_load(1)
        a_round(1, xts[1])
        a_transpose(1, xts[1])
        xts[3] = a_load(3, nc.gpsimd)
        a_round(2, xts[2])
        a_transpose(2, xts[2])
        a_round(3, xts[3])
        a_transpose(3, xts[3])

        b_phase(0, preloaded=2)  # last n iter also emits wave-0 requant chains
        emit_wd_load(0)          # prefetch down weights for D0
        emit_gu_load(0)          # prefetch gate/up block 0 for B1

        # wave-1 quant split into pieces, one per D0 n-iteration
        pieces = []
        for g in (4, 5, 6, 7):
            def mk_load(g=g):
                xts[g] = a_load(g, nc.gpsimd)
            def mk_round(g=g):
                a_round(g, xts[g])
            def mk_tr(g=g):
                a_transpose(g, xts[g])
            pieces += [mk_load, mk_round, mk_tr]

        def d0_hook(n):
            if n < len(pieces):
                pieces[n]()

        d_phase(0, hook=d0_hook)
        b_phase(1, preloaded=1)
        emit_wd_load(0)
        d_phase(1)

    if not nc.is_finalized():
        nc.finalize()
    return nc


def _tile4(wT, K, NBn, NB=256, Q=4):
    # [K*128, NBn*NB] -> [NBn, Q, 128, K/Q, NB]: per-partition 4KB contiguous
    # runs (descriptor-efficient without monopolizing single SBUF partitions)
    return np.ascontiguousarray(
        wT.reshape(Q, K // Q, P, NBn, NB).transpose(3, 0, 2, 1, 4))


def _prep_inputs(x, w_gate, s_wgate, w_up, s_wup, w_down, s_wdown):
    bf16 = ml_dtypes.bfloat16
    x_flat = np.ascontiguousarray(x.reshape(-1, H)).astype(bf16)
    wgt = _tile4(w_gate.astype(bf16).T, H // P, I // 256)  # int-valued: exact
    wut = _tile4(w_up.astype(bf16).T, H // P, I // 256)
    wdt = _tile4(
        (w_down.astype(np.float32) * s_wdown.astype(np.float32)[:, None]).T
        .astype(bf16), I // P, H // 256)                   # fold s_wdown in
    swg = np.ascontiguousarray(s_wgate.reshape(1, I).astype(np.float32))
    swu = np.ascontiguousarray(s_wup.reshape(1, I).astype(np.float32))
    return x_flat, wgt, wut, wdt, swg, swu


def kernel(x, w_gate, s_wgate, w_up, s_wup, w_down, s_wdown,
           inv_gate, inv_up, inv_inter):
    from concourse.bass_utils import run_bass_kernel_spmd

    x_flat, wgt, wut, wdt, swg, swu = _prep_inputs(
        x, w_gate, s_wgate, w_up, s_wup, w_down, s_wdown)

    if "nc" not in _CACHE:
        _CACHE["nc"] = _build()
    nc = _CACHE["nc"]

    in_maps = []
    for c in range(N_CORES):
        in_maps.append({
            "x": np.ascontiguousarray(x_flat[c * T:(c + 1) * T]),
            "wgt": wgt, "wut": wut, "wdt": wdt,
            "swg": swg, "swu": swu,
        })
    res = run_bass_kernel_spmd(nc, in_maps, list(range(N_CORES)))
    _CACHE["last_results"] = res
    _CACHE["last_in_maps"] = in_maps
    out = np.concatenate([res.results[c]["out"] for c in range(N_CORES)], axis=0)
    return out.reshape(x.shape).astype(np.float32)
